# revision 30
# baseline (speedup 1.0000x reference)
"""Bahdanau attention on 8 Trainium2 NeuronCores.

Problem: B=32, S=4096, H=E=512 (fp32)
    q_proj = query @ Wq + bq                              (B, H)
    k_proj = keys @ Wk + bk                               (B, S, H)
    scores = tanh(q_proj[:,None,:] + k_proj) @ v + bv     (B, S)
    scores = where(mask==0, -1e9, scores)
    attn   = softmax(scores, axis=-1)                     (B, S)
    context= einsum('bs,bse->be', attn, keys)             (B, E)
    returns (context, attn)

Sharding: data-parallel over batch, 4 batches per core, no collectives.

Masked positions contribute exactly nothing to either output (the
reference's exp(-1e9 - max) underflows to 0.0 in fp32), so the host
compacts each batch to its unmasked key rows (~50% for the Bernoulli
mask), padded to a fixed tile count chosen from the inputs (min 17
tiles = 2176 slots, ~ +4 sigma above the Binomial(4096, .5) mean); a
wider program is compiled lazily if some batch needs more.

Device algorithm (per core, per batch):
  - bf16 keys arrive in two host-prepared layouts (natural + transposed)
    via plain HWDGE DMAs; natural keys stay resident in SBUF.
  - per s-tile of 128: k_proj matmul (contraction over E on partitions)
    accumulates in PSUM; DVE adds the host-precomputed q_proj+bk bias
    (fp32), ACT computes tanh (bf16), a fused DVE scalar_tensor_tensor
    computes scores = sum_h v*t per partition.
  - softmax without max-subtraction (scores bounded by sum|v| ~ 16; bv
    cancels): e_w = exp(scores) * valid. The denominator's partition
    reduction is a ones-matrix matmul which also broadcasts it.
  - context = (e_w @ keys) / denom via PSUM-accumulated matmuls over the
    resident natural-layout keys tiles.
"""

import numpy as np
import ml_dtypes

B, S, H, E = 32, 4096, 512, 512
NCORES = 8
B_LOC = B // NCORES          # 4 batches per core
P = 128                      # SBUF partitions
EC = E // P                  # 4 e-chunks of 128

NJ_MIN = 17                  # compact capacity floor (2176 slots)
NJ_MAX = 20                  # SBUF limit for the resident-keys device path
LOAD_CHUNK = 6               # s-tiles per load sub-DMA

_CACHE = {}


def _build_nc(nj):
    """Build + compile the per-core program for nj s-tiles per batch."""
    from contextlib import ExitStack

    import concourse.tile as tile
    from concourse import bacc, mybir

    f32 = mybir.dt.float32
    bf16 = mybir.dt.bfloat16
    Alu = mybir.AluOpType
    Act = mybir.ActivationFunctionType

    nc = bacc.Bacc("TRN2", target_bir_lowering=False, debug=False)

    # host-prepared bf16 keys in both layouts:
    #   keys_nat[b, p, jj, e] = keys_c[b, jj*128 + p, e]
    #   keys_t[b, p, jj*EC+c, s] = keys_c[b, jj*128 + s, c*128 + p]
    kn_d = nc.dram_tensor(
        "keys_nat", [B_LOC, P, nj, E], bf16, kind="ExternalInput"
    ).ap()
    kt_d = nc.dram_tensor(
        "keys_t", [B_LOC, P, nj * EC, P], bf16, kind="ExternalInput"
    ).ap()
    qb_d = nc.dram_tensor("qb_bc", [B_LOC, P, H], f32, kind="ExternalInput").ap()
    v_d = nc.dram_tensor("v_bc", [P, H], bf16, kind="ExternalInput").ap()
    wk_d = nc.dram_tensor("wk_t", [P, EC, H], bf16, kind="ExternalInput").ap()
    mf_d = nc.dram_tensor("maskf", [B_LOC, P, nj], f32, kind="ExternalInput").ap()
    ctx_d = nc.dram_tensor("ctx", [B_LOC, E], f32, kind="ExternalOutput").ap()
    attn_d = nc.dram_tensor("attn", [B_LOC, P, nj], f32, kind="ExternalOutput").ap()

    # first chunk small so the first kproj starts ASAP
    bounds = [0, 2]
    while bounds[-1] < nj:
        bounds.append(min(bounds[-1] + LOAD_CHUNK, nj))
    chunks = list(zip(bounds[:-1], bounds[1:]))

    with tile.TileContext(nc) as tc:
        with ExitStack() as ctx:
            consts = ctx.enter_context(tc.tile_pool(name="consts", bufs=1))
            knat_p = ctx.enter_context(tc.tile_pool(name="knat", bufs=B_LOC))
            kT_p = ctx.enter_context(tc.tile_pool(name="keysT", bufs=3))
            z_p = ctx.enter_context(tc.tile_pool(name="z", bufs=4))
            t_p = ctx.enter_context(tc.tile_pool(name="t", bufs=4))
            tv_p = ctx.enter_context(tc.tile_pool(name="tv", bufs=4))
            sc_p = ctx.enter_context(tc.tile_pool(name="scores", bufs=B_LOC))
            s2_p = ctx.enter_context(tc.tile_pool(name="stage2", bufs=2))
            kp_ps = ctx.enter_context(tc.tile_pool(name="kp_ps", bufs=2, space="PSUM"))
            d_ps = ctx.enter_context(tc.tile_pool(name="d_ps", bufs=2, space="PSUM"))
            c_ps = ctx.enter_context(tc.tile_pool(name="c_ps", bufs=2, space="PSUM"))

            # wk is the only const on the sync ring (kproj needs it first);
            # the rest go on the scalar ring, ahead of the knat loads.
            wk_sb = consts.tile([P, EC, H], bf16)
            nc.sync.dma_start(wk_sb[:], wk_d[:])
            qb_sb = consts.tile([P, B_LOC, H], f32)
            nc.scalar.dma_start(qb_sb[:], qb_d.rearrange("b p h -> p b h"))
            v_sb = consts.tile([P, H], bf16)
            nc.scalar.dma_start(v_sb[:], v_d[:])
            mf_sb = consts.tile([P, B_LOC, nj], f32)
            nc.scalar.dma_start(mf_sb[:], mf_d.rearrange("b p j -> p b j"))
            ones_sb = consts.tile([P, P], f32)
            nc.vector.memset(ones_sb[:], 1.0)

            for b in range(B_LOC):
                # ktile (needed first, by kproj) on the sync HWDGE ring,
                # knat (needed last, by context) on the scalar ring.
                ktile = kT_p.tile([P, nj * EC, P], bf16)
                for c0, c1 in chunks:
                    nc.sync.dma_start(
                        ktile[:, c0 * EC : c1 * EC, :],
                        kt_d[b, :, c0 * EC : c1 * EC, :],
                    )
                knat = knat_p.tile([P, nj, E], bf16)
                for c0, c1 in chunks:
                    nc.scalar.dma_start(
                        knat[:, c0:c1, :], kn_d[b, :, c0:c1, :]
                    )

                # process s-tiles in pairs: one 2-bank PSUM tile, one z-add
                # and one tanh per pair (amortizes PSUM access + op overhead)
                sc_b = sc_p.tile([P, nj], f32)
                pairs = [(j, min(j + 2, nj)) for j in range(0, nj, 2)]
                for j0, j1 in pairs:
                    w = j1 - j0
                    kp = kp_ps.tile([P, 2, H], f32)
                    for jj in range(w):
                        for c in range(EC):
                            nc.tensor.matmul(
                                kp[:, jj, :],
                                ktile[:, (j0 + jj) * EC + c, :],
                                wk_sb[:, c, :],
                                start=(c == 0),
                                stop=(c == EC - 1),
                            )
                    z = z_p.tile([P, 2, H], f32)
                    nc.vector.tensor_tensor(
                        out=z[:, :w, :],
                        in0=kp[:, :w, :],
                        in1=qb_sb[:, b, None, :].to_broadcast([P, w, H]),
                        op=Alu.add,
                    )
                    t = t_p.tile([P, 2, H], bf16)
                    nc.scalar.activation(t[:, :w, :], z[:, :w, :], Act.Tanh)
                    for jj in range(w):
                        tv = tv_p.tile([P, H], bf16)
                        nc.vector.scalar_tensor_tensor(
                            out=tv[:],
                            in0=t[:, jj, :],
                            scalar=1.0,
                            in1=v_sb[:],
                            op0=Alu.mult,
                            op1=Alu.mult,
                            accum_out=sc_b[:, j0 + jj : j0 + jj + 1],
                        )

                # stage 2: masked softmax + context
                esc = s2_p.tile([P, nj], f32)
                nc.scalar.activation(esc[:], sc_b[:], Act.Exp)
                ew = s2_p.tile([P, nj], f32)
                colsum = s2_p.tile([P, 1], f32)
                nc.vector.scalar_tensor_tensor(
                    out=ew[:],
                    in0=esc[:],
                    scalar=1.0,
                    in1=mf_sb[:, b, :],
                    op0=Alu.mult,
                    op1=Alu.mult,
                    accum_out=colsum[:],
                )
                ewb = s2_p.tile([P, nj], bf16)
                nc.scalar.copy(out=ewb[:], in_=ew[:])
                pc = c_ps.tile([1, E], f32)
                for j in range(nj):
                    nc.tensor.matmul(
                        pc[:],
                        ewb[:, j : j + 1],
                        knat[:, j, :],
                        start=(j == 0),
                        stop=(j == nj - 1),
                    )
                # partition-reduce the denominator; the ones matmul also
                # broadcasts it to all 128 partitions
                pd = d_ps.tile([P, 1], f32)
                nc.tensor.matmul(pd[:], ones_sb[:], colsum[:], start=True, stop=True)
                rd = s2_p.tile([P, 1], f32)
                nc.vector.reciprocal(rd[:], pd[:])
                # normalization scales via ACT's per-partition scale operand
                attn_sb = s2_p.tile([P, nj], f32)
                nc.scalar.activation(attn_sb[:], ew[:], Act.Copy, scale=rd[:])
                nc.sync.dma_start(attn_d[b], attn_sb[:])
                cs = s2_p.tile([1, E], f32)
                nc.scalar.activation(cs[:], pc[:], Act.Copy, scale=rd[0:1, :])
                nc.sync.dma_start(ctx_d[b : b + 1, :], cs[:])

    nc.compile()
    return nc


def _get_nc(nj):
    key = ("nc", nj)
    if key not in _CACHE:
        _CACHE[key] = _build_nc(nj)
    return _CACHE[key]


def _key_layouts(karr, nj):
    """karr: (B, nj*128, E) bf16 -> (keys_nat, keys_t) device layouts."""
    k5 = karr.reshape(B, nj, P, EC, P)
    keys_nat = np.ascontiguousarray(k5.transpose(0, 2, 1, 3, 4)).reshape(
        B, P, nj, E
    )
    keys_t = np.ascontiguousarray(k5.transpose(0, 4, 1, 3, 2)).reshape(
        B, P, nj * EC, P
    )
    return keys_nat, keys_t


def _prepare(query, keys, mask, Wq, bq, Wk, bk, v, bv):
    """Host-side prep: compact by mask, shard over batch, precompute small
    tensors. Returns (in_maps, idx_list, nj)."""
    query = np.asarray(query, dtype=np.float32)
    keys = np.asarray(keys, dtype=np.float32)
    mask = np.asarray(mask)
    Wq = np.asarray(Wq, dtype=np.float32)
    bq = np.asarray(bq, dtype=np.float32)
    Wk = np.asarray(Wk, dtype=np.float32)
    bk = np.asarray(bk, dtype=np.float32)
    v = np.asarray(v, dtype=np.float32)

    idx_list = [np.flatnonzero(mask[b]) for b in range(B)]
    nmax = max(len(i) for i in idx_list)
    nj = max(NJ_MIN, -(-nmax // P))
    if nj > NJ_MAX:
        return None, idx_list, nj   # caller falls back to host math

    sc_len = nj * P
    karr = np.zeros((B, sc_len, E), dtype=ml_dtypes.bfloat16)
    validf = np.zeros((B, sc_len), dtype=np.float32)
    for b in range(B):
        idx = idx_list[b]
        karr[b, : len(idx)] = keys[b, idx].astype(ml_dtypes.bfloat16)
        validf[b, : len(idx)] = 1.0
    keys_nat, keys_t = _key_layouts(karr, nj)

    # combined per-(b,h) bias: q_proj + bk  (bv cancels in softmax)
    qb = query @ Wq + bq + bk                               # (B, H)
    # Wk rows chunk-major to match the transposed keys layout
    wk_t = np.ascontiguousarray(
        Wk.reshape(EC, P, H).transpose(1, 0, 2)
    ).astype(ml_dtypes.bfloat16)
    v_bc = np.ascontiguousarray(
        np.broadcast_to(v[None, :], (P, H))
    ).astype(ml_dtypes.bfloat16)
    # maskf[b, p, j] = validf[b, j*128+p]
    maskf = np.ascontiguousarray(
        validf.reshape(B, nj, P).transpose(0, 2, 1)
    ).astype(np.float32)

    in_maps = []
    for i in range(NCORES):
        sl = slice(i * B_LOC, (i + 1) * B_LOC)
        qb_bc = np.ascontiguousarray(
            np.broadcast_to(qb[sl][:, None, :], (B_LOC, P, H))
        ).astype(np.float32)
        in_maps.append(
            {
                "keys_nat": np.ascontiguousarray(keys_nat[sl]),
                "keys_t": np.ascontiguousarray(keys_t[sl]),
                "qb_bc": qb_bc,
                "v_bc": v_bc,
                "wk_t": wk_t,
                "maskf": np.ascontiguousarray(maskf[sl]),
            }
        )
    return in_maps, idx_list, nj


def _gather(results, idx_list):
    context = np.empty((B, E), dtype=np.float32)
    attn = np.zeros((B, S), dtype=np.float32)
    for i in range(NCORES):
        ctx_i = np.asarray(results[i]["ctx"])          # (B_LOC, E)
        attn_i = np.asarray(results[i]["attn"])        # (B_LOC, P, nj)
        for b in range(B_LOC):
            gb = i * B_LOC + b
            context[gb] = ctx_i[b]
            idx = idx_list[gb]
            flat = attn_i[b].T.reshape(-1)             # s_c order
            attn[gb, idx] = flat[: len(idx)]
    return context, attn


def _host_reference(query, keys, mask, Wq, bq, Wk, bk, v, bv):
    """Exact host fallback for pathological (denser-than-spec) masks."""
    query, keys = query.astype(np.float64), keys.astype(np.float64)
    q_proj = (query @ Wq.astype(np.float64) + bq)[:, None, :]
    k_proj = np.einsum("bse,eh->bsh", keys, Wk.astype(np.float64)) + bk
    scores = np.einsum("bsh,h->bs", np.tanh(q_proj + k_proj), v.astype(np.float64))
    scores = scores + float(bv)
    scores = np.where(np.asarray(mask) == 0, -1e9, scores)
    m = scores.max(axis=-1, keepdims=True)
    e = np.exp(scores - m)
    attn = e / e.sum(axis=-1, keepdims=True)
    context = np.einsum("bs,bse->be", attn, keys)
    return context.astype(np.float32), attn.astype(np.float32)


def run(inputs, trace=False, tmpdir=None):
    """Run on all 8 cores; returns ((context, attn), BassKernelResults)."""
    from concourse.bass_utils import run_bass_kernel_spmd

    in_maps, idx_list, nj = _prepare(**inputs)
    if in_maps is None:
        return _host_reference(**inputs), None
    nc = _get_nc(nj)
    res = run_bass_kernel_spmd(
        nc, in_maps, list(range(NCORES)), trace=trace, tmpdir=tmpdir
    )
    return _gather(res.results, idx_list), res


def kernel(query, keys, mask, Wq, bq, Wk, bk, v, bv):
    (context, attn), _ = run(
        dict(query=query, keys=keys, mask=mask, Wq=Wq, bq=bq,
             Wk=Wk, bk=bk, v=v, bv=bv)
    )
    return context, attn


# revision 32
# speedup vs baseline: 1.0894x; 1.0894x over previous
"""Bahdanau attention on 8 Trainium2 NeuronCores.

Problem: B=32, S=4096, H=E=512 (fp32)
    q_proj = query @ Wq + bq                              (B, H)
    k_proj = keys @ Wk + bk                               (B, S, H)
    scores = tanh(q_proj[:,None,:] + k_proj) @ v + bv     (B, S)
    scores = where(mask==0, -1e9, scores)
    attn   = softmax(scores, axis=-1)                     (B, S)
    context= einsum('bs,bse->be', attn, keys)             (B, E)
    returns (context, attn)

Sharding: data-parallel over batch, 4 batches per core, no collectives.

Masked positions contribute exactly nothing to either output (the
reference's exp(-1e9 - max) underflows to 0.0 in fp32), so the host
compacts each batch to its unmasked key rows (~50% for the Bernoulli
mask), padded to a fixed tile count chosen from the inputs (min 17
tiles = 2176 slots, ~ +4 sigma above the Binomial(4096, .5) mean); a
wider program is compiled lazily if some batch needs more.

Device algorithm (per core, per batch):
  - bf16 keys arrive in two host-prepared layouts (natural + transposed)
    via plain HWDGE DMAs; natural keys stay resident in SBUF.
  - per s-tile of 128: k_proj matmul (contraction over E on partitions)
    accumulates in PSUM; DVE adds the host-precomputed q_proj+bk bias
    (fp32), ACT computes tanh (bf16), a fused DVE scalar_tensor_tensor
    computes scores = sum_h v*t per partition.
  - softmax without max-subtraction (scores bounded by sum|v| ~ 16; bv
    cancels): e_w = exp(scores) * valid. The denominator's partition
    reduction is a ones-matrix matmul which also broadcasts it.
  - context = (e_w @ keys) / denom via PSUM-accumulated matmuls over the
    resident natural-layout keys tiles.
"""

import numpy as np
import ml_dtypes

B, S, H, E = 32, 4096, 512, 512
NCORES = 8
B_LOC = B // NCORES          # 4 batches per core
P = 128                      # SBUF partitions
EC = E // P                  # 4 e-chunks of 128

NJ_MIN = 17                  # compact capacity floor (2176 slots)
NJ_MAX = 20                  # SBUF limit for the resident-keys device path
LOAD_CHUNK = 6               # s-tiles per load sub-DMA

_CACHE = {}


def _build_nc(nj):
    """Build + compile the per-core program for nj s-tiles per batch."""
    from contextlib import ExitStack

    import concourse.tile as tile
    from concourse import bacc, mybir

    f32 = mybir.dt.float32
    bf16 = mybir.dt.bfloat16
    Alu = mybir.AluOpType
    Act = mybir.ActivationFunctionType

    nc = bacc.Bacc("TRN2", target_bir_lowering=False, debug=False)

    # host-prepared bf16 keys in both layouts:
    #   keys_nat[b, p, jj, e] = keys_c[b, jj*128 + p, e]
    #   keys_t[b, p, jj*EC+c, s] = keys_c[b, jj*128 + s, c*128 + p]
    kn_d = nc.dram_tensor(
        "keys_nat", [B_LOC, P, nj, E], bf16, kind="ExternalInput"
    ).ap()
    kt_d = nc.dram_tensor(
        "keys_t", [B_LOC, P, nj * EC, P], bf16, kind="ExternalInput"
    ).ap()
    qb_d = nc.dram_tensor("qb_bc", [B_LOC, P, H], f32, kind="ExternalInput").ap()
    v_d = nc.dram_tensor("v_bc", [P, H], bf16, kind="ExternalInput").ap()
    wk_d = nc.dram_tensor("wk_t", [P, EC, H], bf16, kind="ExternalInput").ap()
    mf_d = nc.dram_tensor("maskf", [B_LOC, P, nj], f32, kind="ExternalInput").ap()
    ctx_d = nc.dram_tensor("ctx", [B_LOC, E], f32, kind="ExternalOutput").ap()
    attn_d = nc.dram_tensor("attn", [B_LOC, P, nj], f32, kind="ExternalOutput").ap()

    # first chunk small so the first kproj starts ASAP
    bounds = [0, 2]
    while bounds[-1] < nj:
        bounds.append(min(bounds[-1] + LOAD_CHUNK, nj))
    chunks = list(zip(bounds[:-1], bounds[1:]))

    with tile.TileContext(nc) as tc:
        with ExitStack() as ctx:
            consts = ctx.enter_context(tc.tile_pool(name="consts", bufs=1))
            knat_p = ctx.enter_context(tc.tile_pool(name="knat", bufs=B_LOC))
            kT_p = ctx.enter_context(tc.tile_pool(name="keysT", bufs=3))
            z_p = ctx.enter_context(tc.tile_pool(name="z", bufs=4))
            t_p = ctx.enter_context(tc.tile_pool(name="t", bufs=4))
            tv_p = ctx.enter_context(tc.tile_pool(name="tv", bufs=4))
            sc_p = ctx.enter_context(tc.tile_pool(name="scores", bufs=B_LOC))
            s2_p = ctx.enter_context(tc.tile_pool(name="stage2", bufs=2))
            kp_ps = ctx.enter_context(tc.tile_pool(name="kp_ps", bufs=3, space="PSUM"))
            d_ps = ctx.enter_context(tc.tile_pool(name="d_ps", bufs=1, space="PSUM"))
            c_ps = ctx.enter_context(tc.tile_pool(name="c_ps", bufs=1, space="PSUM"))

            # wk is the only const on the sync ring (kproj needs it first);
            # the rest go on the scalar ring, ahead of the knat loads.
            wk_sb = consts.tile([P, EC, H], bf16)
            for c in range(EC):
                nc.sync.dma_start(wk_sb[:, c, :], wk_d[:, c, :])
            qb_sb = consts.tile([P, B_LOC, H], f32)
            nc.scalar.dma_start(qb_sb[:], qb_d.rearrange("b p h -> p b h"))
            v_sb = consts.tile([P, H], bf16)
            nc.scalar.dma_start(v_sb[:], v_d[:])
            mf_sb = consts.tile([P, B_LOC, nj], f32)
            nc.scalar.dma_start(mf_sb[:], mf_d.rearrange("b p j -> p b j"))
            ones_sb = consts.tile([P, P], f32)
            nc.vector.memset(ones_sb[:], 1.0)

            for b in range(B_LOC):
                # ktile (needed first, by kproj) on the sync HWDGE ring,
                # knat (needed last, by context) on the scalar ring.
                ktile = kT_p.tile([P, nj * EC, P], bf16)
                for c0, c1 in chunks:
                    nc.sync.dma_start(
                        ktile[:, c0 * EC : c1 * EC, :],
                        kt_d[b, :, c0 * EC : c1 * EC, :],
                    )
                knat = knat_p.tile([P, nj, E], bf16)
                for c0, c1 in chunks:
                    nc.scalar.dma_start(
                        knat[:, c0:c1, :], kn_d[b, :, c0:c1, :]
                    )

                # process s-tiles in pairs: one 2-bank PSUM tile, one z-add
                # and one tanh per pair (amortizes PSUM access + op overhead)
                sc_b = sc_p.tile([P, nj], f32)
                pairs = [(j, min(j + 2, nj)) for j in range(0, nj, 2)]
                for j0, j1 in pairs:
                    w = j1 - j0
                    kp = kp_ps.tile([P, 2, H], f32)
                    for jj in range(w):
                        for c in range(EC):
                            nc.tensor.matmul(
                                kp[:, jj, :],
                                ktile[:, (j0 + jj) * EC + c, :],
                                wk_sb[:, c, :],
                                start=(c == 0),
                                stop=(c == EC - 1),
                            )
                    z = z_p.tile([P, 2, H], f32)
                    nc.vector.tensor_tensor(
                        out=z[:, :w, :],
                        in0=kp[:, :w, :],
                        in1=qb_sb[:, b, None, :].to_broadcast([P, w, H]),
                        op=Alu.add,
                    )
                    t = t_p.tile([P, 2, H], bf16)
                    nc.scalar.activation(t[:, :w, :], z[:, :w, :], Act.Tanh)
                    for jj in range(w):
                        tv = tv_p.tile([P, H], bf16)
                        nc.vector.scalar_tensor_tensor(
                            out=tv[:],
                            in0=t[:, jj, :],
                            scalar=1.0,
                            in1=v_sb[:],
                            op0=Alu.mult,
                            op1=Alu.mult,
                            accum_out=sc_b[:, j0 + jj : j0 + jj + 1],
                        )

                # stage 2: masked softmax + context
                esc = s2_p.tile([P, nj], f32)
                nc.scalar.activation(esc[:], sc_b[:], Act.Exp)
                ew = s2_p.tile([P, nj], f32)
                colsum = s2_p.tile([P, 1], f32)
                nc.vector.scalar_tensor_tensor(
                    out=ew[:],
                    in0=esc[:],
                    scalar=1.0,
                    in1=mf_sb[:, b, :],
                    op0=Alu.mult,
                    op1=Alu.mult,
                    accum_out=colsum[:],
                )
                ewb = s2_p.tile([P, nj], bf16)
                nc.scalar.copy(out=ewb[:], in_=ew[:])
                pc = c_ps.tile([1, E], f32)
                for j in range(nj):
                    nc.tensor.matmul(
                        pc[:],
                        ewb[:, j : j + 1],
                        knat[:, j, :],
                        start=(j == 0),
                        stop=(j == nj - 1),
                    )
                # partition-reduce the denominator; the ones matmul also
                # broadcasts it to all 128 partitions
                pd = d_ps.tile([P, 1], f32)
                nc.tensor.matmul(pd[:], ones_sb[:], colsum[:], start=True, stop=True)
                rd = s2_p.tile([P, 1], f32)
                nc.vector.reciprocal(rd[:], pd[:])
                # normalization scales via ACT's per-partition scale operand
                attn_sb = s2_p.tile([P, nj], f32)
                nc.scalar.activation(attn_sb[:], ew[:], Act.Copy, scale=rd[:])
                nc.sync.dma_start(attn_d[b], attn_sb[:])
                cs = s2_p.tile([1, E], f32)
                nc.scalar.activation(cs[:], pc[:], Act.Copy, scale=rd[0:1, :])
                nc.sync.dma_start(ctx_d[b : b + 1, :], cs[:])

    nc.compile()
    return nc


def _get_nc(nj):
    key = ("nc", nj)
    if key not in _CACHE:
        _CACHE[key] = _build_nc(nj)
    return _CACHE[key]


def _key_layouts(karr, nj):
    """karr: (B, nj*128, E) bf16 -> (keys_nat, keys_t) device layouts."""
    k5 = karr.reshape(B, nj, P, EC, P)
    keys_nat = np.ascontiguousarray(k5.transpose(0, 2, 1, 3, 4)).reshape(
        B, P, nj, E
    )
    keys_t = np.ascontiguousarray(k5.transpose(0, 4, 1, 3, 2)).reshape(
        B, P, nj * EC, P
    )
    return keys_nat, keys_t


def _prepare(query, keys, mask, Wq, bq, Wk, bk, v, bv):
    """Host-side prep: compact by mask, shard over batch, precompute small
    tensors. Returns (in_maps, idx_list, nj)."""
    query = np.asarray(query, dtype=np.float32)
    keys = np.asarray(keys, dtype=np.float32)
    mask = np.asarray(mask)
    Wq = np.asarray(Wq, dtype=np.float32)
    bq = np.asarray(bq, dtype=np.float32)
    Wk = np.asarray(Wk, dtype=np.float32)
    bk = np.asarray(bk, dtype=np.float32)
    v = np.asarray(v, dtype=np.float32)

    idx_list = [np.flatnonzero(mask[b]) for b in range(B)]
    nmax = max(len(i) for i in idx_list)
    nj = max(NJ_MIN, -(-nmax // P))
    if nj > NJ_MAX:
        return None, idx_list, nj   # caller falls back to host math

    sc_len = nj * P
    karr = np.zeros((B, sc_len, E), dtype=ml_dtypes.bfloat16)
    validf = np.zeros((B, sc_len), dtype=np.float32)
    for b in range(B):
        idx = idx_list[b]
        karr[b, : len(idx)] = keys[b, idx].astype(ml_dtypes.bfloat16)
        validf[b, : len(idx)] = 1.0
    keys_nat, keys_t = _key_layouts(karr, nj)

    # combined per-(b,h) bias: q_proj + bk  (bv cancels in softmax)
    qb = query @ Wq + bq + bk                               # (B, H)
    # Wk rows chunk-major to match the transposed keys layout
    wk_t = np.ascontiguousarray(
        Wk.reshape(EC, P, H).transpose(1, 0, 2)
    ).astype(ml_dtypes.bfloat16)
    v_bc = np.ascontiguousarray(
        np.broadcast_to(v[None, :], (P, H))
    ).astype(ml_dtypes.bfloat16)
    # maskf[b, p, j] = validf[b, j*128+p]
    maskf = np.ascontiguousarray(
        validf.reshape(B, nj, P).transpose(0, 2, 1)
    ).astype(np.float32)

    in_maps = []
    for i in range(NCORES):
        sl = slice(i * B_LOC, (i + 1) * B_LOC)
        qb_bc = np.ascontiguousarray(
            np.broadcast_to(qb[sl][:, None, :], (B_LOC, P, H))
        ).astype(np.float32)
        in_maps.append(
            {
                "keys_nat": np.ascontiguousarray(keys_nat[sl]),
                "keys_t": np.ascontiguousarray(keys_t[sl]),
                "qb_bc": qb_bc,
                "v_bc": v_bc,
                "wk_t": wk_t,
                "maskf": np.ascontiguousarray(maskf[sl]),
            }
        )
    return in_maps, idx_list, nj


def _gather(results, idx_list):
    context = np.empty((B, E), dtype=np.float32)
    attn = np.zeros((B, S), dtype=np.float32)
    for i in range(NCORES):
        ctx_i = np.asarray(results[i]["ctx"])          # (B_LOC, E)
        attn_i = np.asarray(results[i]["attn"])        # (B_LOC, P, nj)
        for b in range(B_LOC):
            gb = i * B_LOC + b
            context[gb] = ctx_i[b]
            idx = idx_list[gb]
            flat = attn_i[b].T.reshape(-1)             # s_c order
            attn[gb, idx] = flat[: len(idx)]
    return context, attn


def _host_reference(query, keys, mask, Wq, bq, Wk, bk, v, bv):
    """Exact host fallback for pathological (denser-than-spec) masks."""
    query, keys = query.astype(np.float64), keys.astype(np.float64)
    q_proj = (query @ Wq.astype(np.float64) + bq)[:, None, :]
    k_proj = np.einsum("bse,eh->bsh", keys, Wk.astype(np.float64)) + bk
    scores = np.einsum("bsh,h->bs", np.tanh(q_proj + k_proj), v.astype(np.float64))
    scores = scores + float(bv)
    scores = np.where(np.asarray(mask) == 0, -1e9, scores)
    m = scores.max(axis=-1, keepdims=True)
    e = np.exp(scores - m)
    attn = e / e.sum(axis=-1, keepdims=True)
    context = np.einsum("bs,bse->be", attn, keys)
    return context.astype(np.float32), attn.astype(np.float32)


def run(inputs, trace=False, tmpdir=None):
    """Run on all 8 cores; returns ((context, attn), BassKernelResults)."""
    from concourse.bass_utils import run_bass_kernel_spmd

    in_maps, idx_list, nj = _prepare(**inputs)
    if in_maps is None:
        return _host_reference(**inputs), None
    nc = _get_nc(nj)
    res = run_bass_kernel_spmd(
        nc, in_maps, list(range(NCORES)), trace=trace, tmpdir=tmpdir
    )
    return _gather(res.results, idx_list), res


def kernel(query, keys, mask, Wq, bq, Wk, bk, v, bv):
    (context, attn), _ = run(
        dict(query=query, keys=keys, mask=mask, Wq=Wq, bq=bq,
             Wk=Wk, bk=bk, v=v, bv=bv)
    )
    return context, attn


# revision 33
# speedup vs baseline: 1.1056x; 1.0148x over previous
"""Bahdanau attention on 8 Trainium2 NeuronCores.

Problem: B=32, S=4096, H=E=512 (fp32)
    q_proj = query @ Wq + bq                              (B, H)
    k_proj = keys @ Wk + bk                               (B, S, H)
    scores = tanh(q_proj[:,None,:] + k_proj) @ v + bv     (B, S)
    scores = where(mask==0, -1e9, scores)
    attn   = softmax(scores, axis=-1)                     (B, S)
    context= einsum('bs,bse->be', attn, keys)             (B, E)
    returns (context, attn)

Sharding: data-parallel over batch, 4 batches per core, no collectives.

Masked positions contribute exactly nothing to either output (the
reference's exp(-1e9 - max) underflows to 0.0 in fp32), so the host
compacts each batch to its unmasked key rows (~50% for the Bernoulli
mask), padded to a fixed tile count chosen from the inputs (min 17
tiles = 2176 slots, ~ +4 sigma above the Binomial(4096, .5) mean); a
wider program is compiled lazily if some batch needs more.

Device algorithm (per core, per batch):
  - bf16 keys arrive in two host-prepared layouts (natural + transposed)
    via plain HWDGE DMAs; natural keys stay resident in SBUF.
  - per s-tile of 128: k_proj matmul (contraction over E on partitions)
    accumulates in PSUM; DVE adds the host-precomputed q_proj+bk bias
    (fp32), ACT computes tanh (bf16), a fused DVE scalar_tensor_tensor
    computes scores = sum_h v*t per partition.
  - softmax without max-subtraction (scores bounded by sum|v| ~ 16; bv
    cancels): e_w = exp(scores) * valid. The denominator's partition
    reduction is a ones-matrix matmul which also broadcasts it.
  - context = (e_w @ keys) / denom via PSUM-accumulated matmuls over the
    resident natural-layout keys tiles.
"""

import numpy as np
import ml_dtypes

B, S, H, E = 32, 4096, 512, 512
NCORES = 8
B_LOC = B // NCORES          # 4 batches per core
P = 128                      # SBUF partitions
EC = E // P                  # 4 e-chunks of 128

NJ_MIN = 17                  # compact capacity floor (2176 slots)
NJ_MAX = 20                  # SBUF limit for the resident-keys device path
LOAD_CHUNK = 6               # s-tiles per load sub-DMA

_CACHE = {}


def _build_nc(nj):
    """Build + compile the per-core program for nj s-tiles per batch."""
    from contextlib import ExitStack

    import concourse.tile as tile
    from concourse import bacc, mybir

    f32 = mybir.dt.float32
    bf16 = mybir.dt.bfloat16
    Alu = mybir.AluOpType
    Act = mybir.ActivationFunctionType

    nc = bacc.Bacc("TRN2", target_bir_lowering=False, debug=False)

    # host-prepared bf16 keys in both layouts:
    #   keys_nat[b, p, jj, e] = keys_c[b, jj*128 + p, e]
    #   keys_t[b, p, jj*EC+c, s] = keys_c[b, jj*128 + s, c*128 + p]
    kn_d = nc.dram_tensor(
        "keys_nat", [B_LOC, P, nj, E], bf16, kind="ExternalInput"
    ).ap()
    kt_d = nc.dram_tensor(
        "keys_t", [B_LOC, P, nj * EC, P], bf16, kind="ExternalInput"
    ).ap()
    qb_d = nc.dram_tensor("qb_bc", [B_LOC, P, H], f32, kind="ExternalInput").ap()
    v_d = nc.dram_tensor("v_bc", [P, H], bf16, kind="ExternalInput").ap()
    wk_d = nc.dram_tensor("wk_t", [P, EC, H], bf16, kind="ExternalInput").ap()
    mf_d = nc.dram_tensor("maskf", [B_LOC, P, nj], f32, kind="ExternalInput").ap()
    ctx_d = nc.dram_tensor("ctx", [B_LOC, E], f32, kind="ExternalOutput").ap()
    attn_d = nc.dram_tensor("attn", [B_LOC, P, nj], f32, kind="ExternalOutput").ap()

    # first chunk small so the first kproj starts ASAP
    bounds = [0, 2]
    while bounds[-1] < nj:
        bounds.append(min(bounds[-1] + LOAD_CHUNK, nj))
    chunks = list(zip(bounds[:-1], bounds[1:]))

    with tile.TileContext(nc) as tc:
        with ExitStack() as ctx:
            consts = ctx.enter_context(tc.tile_pool(name="consts", bufs=1))
            knat_p = ctx.enter_context(tc.tile_pool(name="knat", bufs=B_LOC))
            kT_p = ctx.enter_context(tc.tile_pool(name="keysT", bufs=3))
            z_p = ctx.enter_context(tc.tile_pool(name="z", bufs=6))
            t_p = ctx.enter_context(tc.tile_pool(name="t", bufs=6))
            tv_p = ctx.enter_context(tc.tile_pool(name="tv", bufs=4))
            sc_p = ctx.enter_context(tc.tile_pool(name="scores", bufs=B_LOC))
            s2_p = ctx.enter_context(tc.tile_pool(name="stage2", bufs=2))
            kp_ps = ctx.enter_context(tc.tile_pool(name="kp_ps", bufs=3, space="PSUM"))
            d_ps = ctx.enter_context(tc.tile_pool(name="d_ps", bufs=1, space="PSUM"))
            c_ps = ctx.enter_context(tc.tile_pool(name="c_ps", bufs=1, space="PSUM"))

            # wk is the only const on the sync ring (kproj needs it first);
            # the rest go on the scalar ring, ahead of the knat loads.
            wk_sb = consts.tile([P, EC, H], bf16)
            for c in range(EC):
                nc.sync.dma_start(wk_sb[:, c, :], wk_d[:, c, :])
            qb_sb = consts.tile([P, B_LOC, H], f32)
            nc.scalar.dma_start(qb_sb[:], qb_d.rearrange("b p h -> p b h"))
            v_sb = consts.tile([P, H], bf16)
            nc.scalar.dma_start(v_sb[:], v_d[:])
            mf_sb = consts.tile([P, B_LOC, nj], f32)
            nc.scalar.dma_start(mf_sb[:], mf_d.rearrange("b p j -> p b j"))
            ones_sb = consts.tile([P, P], f32)
            nc.vector.memset(ones_sb[:], 1.0)

            for b in range(B_LOC):
                # ktile (needed first, by kproj) on the sync HWDGE ring,
                # knat (needed last, by context) on the scalar ring.
                ktile = kT_p.tile([P, nj * EC, P], bf16)
                for c0, c1 in chunks:
                    nc.sync.dma_start(
                        ktile[:, c0 * EC : c1 * EC, :],
                        kt_d[b, :, c0 * EC : c1 * EC, :],
                    )
                knat = knat_p.tile([P, nj, E], bf16)
                for c0, c1 in chunks:
                    nc.scalar.dma_start(
                        knat[:, c0:c1, :], kn_d[b, :, c0:c1, :]
                    )

                # process s-tiles in pairs: one 2-bank PSUM tile, one z-add
                # and one tanh per pair (amortizes PSUM access + op overhead)
                sc_b = sc_p.tile([P, nj], f32)
                pairs = [(j, min(j + 2, nj)) for j in range(0, nj, 2)]
                for j0, j1 in pairs:
                    w = j1 - j0
                    kp = kp_ps.tile([P, 2, H], f32)
                    for jj in range(w):
                        for c in range(EC):
                            nc.tensor.matmul(
                                kp[:, jj, :],
                                ktile[:, (j0 + jj) * EC + c, :],
                                wk_sb[:, c, :],
                                start=(c == 0),
                                stop=(c == EC - 1),
                            )
                    z = z_p.tile([P, 2, H], f32)
                    nc.vector.tensor_tensor(
                        out=z[:, :w, :],
                        in0=kp[:, :w, :],
                        in1=qb_sb[:, b, None, :].to_broadcast([P, w, H]),
                        op=Alu.add,
                    )
                    t = t_p.tile([P, 2, H], bf16)
                    nc.scalar.activation(t[:, :w, :], z[:, :w, :], Act.Tanh)
                    for jj in range(w):
                        tv = tv_p.tile([P, H], bf16)
                        nc.vector.scalar_tensor_tensor(
                            out=tv[:],
                            in0=t[:, jj, :],
                            scalar=1.0,
                            in1=v_sb[:],
                            op0=Alu.mult,
                            op1=Alu.mult,
                            accum_out=sc_b[:, j0 + jj : j0 + jj + 1],
                        )

                # stage 2: masked softmax + context
                esc = s2_p.tile([P, nj], f32)
                nc.scalar.activation(esc[:], sc_b[:], Act.Exp)
                ew = s2_p.tile([P, nj], f32)
                colsum = s2_p.tile([P, 1], f32)
                nc.vector.scalar_tensor_tensor(
                    out=ew[:],
                    in0=esc[:],
                    scalar=1.0,
                    in1=mf_sb[:, b, :],
                    op0=Alu.mult,
                    op1=Alu.mult,
                    accum_out=colsum[:],
                )
                ewb = s2_p.tile([P, nj], bf16)
                nc.scalar.copy(out=ewb[:], in_=ew[:])
                pc = c_ps.tile([1, E], f32)
                for j in range(nj):
                    nc.tensor.matmul(
                        pc[:],
                        ewb[:, j : j + 1],
                        knat[:, j, :],
                        start=(j == 0),
                        stop=(j == nj - 1),
                    )
                # partition-reduce the denominator; the ones matmul also
                # broadcasts it to all 128 partitions
                pd = d_ps.tile([P, 1], f32)
                nc.tensor.matmul(pd[:], ones_sb[:], colsum[:], start=True, stop=True)
                rd = s2_p.tile([P, 1], f32)
                nc.vector.reciprocal(rd[:], pd[:])
                # normalization scales via ACT's per-partition scale operand
                attn_sb = s2_p.tile([P, nj], f32)
                nc.scalar.activation(attn_sb[:], ew[:], Act.Copy, scale=rd[:])
                nc.sync.dma_start(attn_d[b], attn_sb[:])
                cs = s2_p.tile([1, E], f32)
                nc.scalar.activation(cs[:], pc[:], Act.Copy, scale=rd[0:1, :])
                nc.sync.dma_start(ctx_d[b : b + 1, :], cs[:])

    nc.compile()
    return nc


def _get_nc(nj):
    key = ("nc", nj)
    if key not in _CACHE:
        _CACHE[key] = _build_nc(nj)
    return _CACHE[key]


def _key_layouts(karr, nj):
    """karr: (B, nj*128, E) bf16 -> (keys_nat, keys_t) device layouts."""
    k5 = karr.reshape(B, nj, P, EC, P)
    keys_nat = np.ascontiguousarray(k5.transpose(0, 2, 1, 3, 4)).reshape(
        B, P, nj, E
    )
    keys_t = np.ascontiguousarray(k5.transpose(0, 4, 1, 3, 2)).reshape(
        B, P, nj * EC, P
    )
    return keys_nat, keys_t


def _prepare(query, keys, mask, Wq, bq, Wk, bk, v, bv):
    """Host-side prep: compact by mask, shard over batch, precompute small
    tensors. Returns (in_maps, idx_list, nj)."""
    query = np.asarray(query, dtype=np.float32)
    keys = np.asarray(keys, dtype=np.float32)
    mask = np.asarray(mask)
    Wq = np.asarray(Wq, dtype=np.float32)
    bq = np.asarray(bq, dtype=np.float32)
    Wk = np.asarray(Wk, dtype=np.float32)
    bk = np.asarray(bk, dtype=np.float32)
    v = np.asarray(v, dtype=np.float32)

    idx_list = [np.flatnonzero(mask[b]) for b in range(B)]
    nmax = max(len(i) for i in idx_list)
    nj = max(NJ_MIN, -(-nmax // P))
    if nj > NJ_MAX:
        return None, idx_list, nj   # caller falls back to host math

    sc_len = nj * P
    karr = np.zeros((B, sc_len, E), dtype=ml_dtypes.bfloat16)
    validf = np.zeros((B, sc_len), dtype=np.float32)
    for b in range(B):
        idx = idx_list[b]
        karr[b, : len(idx)] = keys[b, idx].astype(ml_dtypes.bfloat16)
        validf[b, : len(idx)] = 1.0
    keys_nat, keys_t = _key_layouts(karr, nj)

    # combined per-(b,h) bias: q_proj + bk  (bv cancels in softmax)
    qb = query @ Wq + bq + bk                               # (B, H)
    # Wk rows chunk-major to match the transposed keys layout
    wk_t = np.ascontiguousarray(
        Wk.reshape(EC, P, H).transpose(1, 0, 2)
    ).astype(ml_dtypes.bfloat16)
    v_bc = np.ascontiguousarray(
        np.broadcast_to(v[None, :], (P, H))
    ).astype(ml_dtypes.bfloat16)
    # maskf[b, p, j] = validf[b, j*128+p]
    maskf = np.ascontiguousarray(
        validf.reshape(B, nj, P).transpose(0, 2, 1)
    ).astype(np.float32)

    in_maps = []
    for i in range(NCORES):
        sl = slice(i * B_LOC, (i + 1) * B_LOC)
        qb_bc = np.ascontiguousarray(
            np.broadcast_to(qb[sl][:, None, :], (B_LOC, P, H))
        ).astype(np.float32)
        in_maps.append(
            {
                "keys_nat": np.ascontiguousarray(keys_nat[sl]),
                "keys_t": np.ascontiguousarray(keys_t[sl]),
                "qb_bc": qb_bc,
                "v_bc": v_bc,
                "wk_t": wk_t,
                "maskf": np.ascontiguousarray(maskf[sl]),
            }
        )
    return in_maps, idx_list, nj


def _gather(results, idx_list):
    context = np.empty((B, E), dtype=np.float32)
    attn = np.zeros((B, S), dtype=np.float32)
    for i in range(NCORES):
        ctx_i = np.asarray(results[i]["ctx"])          # (B_LOC, E)
        attn_i = np.asarray(results[i]["attn"])        # (B_LOC, P, nj)
        for b in range(B_LOC):
            gb = i * B_LOC + b
            context[gb] = ctx_i[b]
            idx = idx_list[gb]
            flat = attn_i[b].T.reshape(-1)             # s_c order
            attn[gb, idx] = flat[: len(idx)]
    return context, attn


def _host_reference(query, keys, mask, Wq, bq, Wk, bk, v, bv):
    """Exact host fallback for pathological (denser-than-spec) masks."""
    query, keys = query.astype(np.float64), keys.astype(np.float64)
    q_proj = (query @ Wq.astype(np.float64) + bq)[:, None, :]
    k_proj = np.einsum("bse,eh->bsh", keys, Wk.astype(np.float64)) + bk
    scores = np.einsum("bsh,h->bs", np.tanh(q_proj + k_proj), v.astype(np.float64))
    scores = scores + float(bv)
    scores = np.where(np.asarray(mask) == 0, -1e9, scores)
    m = scores.max(axis=-1, keepdims=True)
    e = np.exp(scores - m)
    attn = e / e.sum(axis=-1, keepdims=True)
    context = np.einsum("bs,bse->be", attn, keys)
    return context.astype(np.float32), attn.astype(np.float32)


def run(inputs, trace=False, tmpdir=None):
    """Run on all 8 cores; returns ((context, attn), BassKernelResults)."""
    from concourse.bass_utils import run_bass_kernel_spmd

    in_maps, idx_list, nj = _prepare(**inputs)
    if in_maps is None:
        return _host_reference(**inputs), None
    nc = _get_nc(nj)
    res = run_bass_kernel_spmd(
        nc, in_maps, list(range(NCORES)), trace=trace, tmpdir=tmpdir
    )
    return _gather(res.results, idx_list), res


def kernel(query, keys, mask, Wq, bq, Wk, bk, v, bv):
    (context, attn), _ = run(
        dict(query=query, keys=keys, mask=mask, Wq=Wq, bq=bq,
             Wk=Wk, bk=bk, v=v, bv=bv)
    )
    return context, attn
